# revision 11
# baseline (speedup 1.0000x reference)
"""HeadQK kernel for trn2: out = segsum_vocab(causal(q @ k.T / 256)) over 8 cores.

Strategy: shard the vocab dimension of the output across the 8 cores.
Core p owns vocab slice [VS*p, VS*(p+1)).  For its slice it needs only the
attention columns s with idx[s] in the slice (~T/8 of them), but all of q.
The output block is produced TRANSPOSED ([VS, T]) so each touched vocab row
is a contiguous DMA; untouched rows remain zero via the runtime's
zero-initialized output buffers.  The segment-sum becomes a small 0/1
group-matrix matmul on the PE array (groups ordered by first source so the
matrix is band-diagonal and zero blocks are skipped); causal masking is an
iota>=s compare on the vector engine.  All matmuls run in float32r.
"""

import math
import sys

import numpy as np

if "/opt/trn_rl_repo" not in sys.path:
    sys.path.insert(0, "/opt/trn_rl_repo")

import concourse.bacc as bacc
import concourse.mybir as mybir
import concourse.tile as tile
from concourse import bass
from concourse.bass_utils import run_bass_kernel_spmd

T, C, D, V = 4096, 1024, 256, 32000
NCORES = 8
VS = V // NCORES        # 4000 vocab slots per core
CH, CW = 8, 512         # t chunks: 8 x 512
NP = 4                  # qT passes, 2 chunks each
CT = C // 128           # 8 contraction tiles
DT = D // 128           # 2 d tiles
F32 = mybir.dt.float32
F32R = mybir.dt.float32r
I32 = mybir.dt.int32
OOB = 10**8


def _build(J, JT, UT, active, masked, gnnz):
    """Build the SPMD program.

    active/masked: dict[(ch, jt)] -> bool
    gnnz: dict[(jt, ut)] -> bool  (G block non-zero on any core)
    """
    nc = bacc.Bacc("TRN2", target_bir_lowering=False, debug=False,
                   num_devices=NCORES)
    xh = nc.dram_tensor("xh", [NP, CT, 128, 2 * CW], F32R, kind="ExternalInput")
    wq = nc.dram_tensor("wq", [128, CT * D], F32R, kind="ExternalInput")
    wk = nc.dram_tensor("wk", [128, CT * D], F32R, kind="ExternalInput")
    xst = nc.dram_tensor("xst", [CT, 128, J], F32R, kind="ExternalInput")
    sadj = nc.dram_tensor("sadj", [128, CH * JT], F32, kind="ExternalInput")
    gct = nc.dram_tensor("gct", [128, JT * UT * 128], F32R, kind="ExternalInput")
    uoff = nc.dram_tensor("uoff", [128, CH * UT], I32, kind="ExternalInput")
    iota = nc.dram_tensor("iota", [128, CW], F32, kind="ExternalInput")
    out = nc.dram_tensor("out", [VS * CH, CW], F32, kind="ExternalOutput")

    with tile.TileContext(nc) as tc:
        with (
            tc.tile_pool(name="const", bufs=1) as cpool,
            tc.tile_pool(name="xbuf", bufs=2) as xpool,
            tc.tile_pool(name="ctm", bufs=2) as mpool,
            tc.tile_pool(name="gout", bufs=2) as gpool,
            tc.tile_pool(name="psq", bufs=4, space="PSUM") as psq,
            tc.tile_pool(name="psc", bufs=2, space="PSUM") as psc,
            tc.tile_pool(name="psg", bufs=2, space="PSUM") as psg,
        ):
            # ---- loads: scalar queue = consts+wk, sync queue = xst+wq ----
            iota_b = cpool.tile([128, CW], F32)
            nc.scalar.dma_start(out=iota_b[:], in_=iota[:])
            sadj_b = cpool.tile([128, CH * JT], F32)
            nc.scalar.dma_start(out=sadj_b[:], in_=sadj[:])
            wk_b = cpool.tile([128, CT * D], F32R)
            nc.scalar.dma_start(out=wk_b[:], in_=wk[:])
            uoff_b = cpool.tile([128, CH * UT], I32)
            nc.scalar.dma_start(out=uoff_b[:], in_=uoff[:])
            gct_b = cpool.tile([128, JT * UT * 128], F32R)
            nc.scalar.dma_start(out=gct_b[:], in_=gct[:])
            xst_b = []
            for c8 in range(CT):
                xt_ = cpool.tile([128, J], F32R, tag=f"xst{c8}")
                nc.sync.dma_start(out=xt_[:], in_=xst[c8])
                xst_b.append(xt_)
            wq_b = cpool.tile([128, CT * D], F32R)
            nc.sync.dma_start(out=wq_b[:], in_=wq[:])

            # ---- precompute causal masks on the (idle) early DVE ----
            maskt = {}
            for ch in range(CH):
                for jt in range(JT):
                    if masked[(ch, jt)]:
                        mk = cpool.tile([128, CW], F32, tag=f"mk{ch}_{jt}")
                        nc.vector.tensor_tensor(
                            out=mk[:], in0=iota_b[:],
                            in1=sadj_b[:, ch * JT + jt:ch * JT + jt + 1]
                            .to_broadcast([128, CW]),
                            op=mybir.AluOpType.is_ge,
                        )
                        maskt[(ch, jt)] = mk

            # ---- kST[d, j] = Wk.T @ x[S].T (c8-outer: weights amortized) ----
            JW = [(j0, min(512, J - j0)) for j0 in range(0, J, 512)]
            kacc = {}
            for d in range(DT):
                for j0, jw in JW:
                    ka = psq.tile([128, jw], F32, tag="qtp")
                    kacc[(d, j0)] = ka
            for c8 in range(CT):
                for d in range(DT):
                    for j0, jw in JW:
                        nc.tensor.matmul(
                            out=kacc[(d, j0)][:],
                            lhsT=wk_b[:, c8 * D + d * 128:c8 * D + (d + 1) * 128],
                            rhs=xst_b[c8][:, j0:j0 + jw],
                            start=(c8 == 0), stop=(c8 == CT - 1),
                        )
            kst = []
            for d in range(DT):
                kt = cpool.tile([128, J], F32R, tag=f"kst{d}")
                for j0, jw in JW:
                    nc.any.tensor_copy(out=kt[:, j0:j0 + jw], in_=kacc[(d, j0)][:])
                kst.append(kt)

            # ---- qT passes (reverse order) interleaved with chunk work ----
            qt_all = []
            for d in range(DT):
                qa_t = cpool.tile([128, T], F32R, tag=f"qt{d}")
                qt_all.append(qa_t)

            def qt_pass(ps):
                xq = []
                for c8 in range(CT):
                    xq_t = xpool.tile([128, 2 * CW], F32R, tag="xq")
                    eng = nc.sync if c8 % 2 == 0 else nc.scalar
                    eng.dma_start(out=xq_t[:], in_=xh[ps, c8])
                    xq.append(xq_t)
                qacc = {}
                for d in range(DT):
                    for cc in range(2):
                        qa = psq.tile([128, CW], F32, tag="qtp")
                        qacc[(d, cc)] = qa
                for c8 in range(CT):
                    for d in range(DT):
                        for cc in range(2):
                            nc.tensor.matmul(
                                out=qacc[(d, cc)][:],
                                lhsT=wq_b[:, c8 * D + d * 128:c8 * D + (d + 1) * 128],
                                rhs=xq[c8][:, cc * CW:(cc + 1) * CW],
                                start=(c8 == 0), stop=(c8 == CT - 1),
                            )
                for d in range(DT):
                    for cc in range(2):
                        t0 = (2 * ps + cc) * CW
                        nc.any.tensor_copy(out=qt_all[d][:, t0:t0 + CW],
                                           in_=qacc[(d, cc)][:])

            def chunk_work(ch):
                act = [jt for jt in range(JT) if active[(ch, jt)]]
                if not act:
                    return
                ctm = {}
                for jt in act:
                    acc = psc.tile([128, CW], F32, tag="ctp")
                    for d in range(DT):
                        nc.tensor.matmul(
                            out=acc[:],
                            lhsT=kst[d][:, jt * 128:(jt + 1) * 128],
                            rhs=qt_all[d][:, ch * CW:(ch + 1) * CW],
                            start=(d == 0), stop=(d == DT - 1),
                        )
                    cm = mpool.tile([128, CW], F32R, tag=f"cm{jt}")
                    if masked[(ch, jt)]:
                        nc.vector.tensor_tensor(
                            out=cm[:], in0=acc[:], in1=maskt[(ch, jt)][:],
                            op=mybir.AluOpType.mult,
                        )
                    else:
                        nc.any.tensor_copy(out=cm[:], in_=acc[:])
                    ctm[jt] = cm

                for ut in range(UT):
                    jts = [jt for jt in act if gnnz[(jt, ut)]]
                    if not jts:
                        continue
                    gacc = psg.tile([128, CW], F32, tag="gp")
                    for i, jt in enumerate(jts):
                        nc.tensor.matmul(
                            out=gacc[:],
                            lhsT=gct_b[:, jt * UT * 128 + ut * 128:
                                       jt * UT * 128 + (ut + 1) * 128],
                            rhs=ctm[jt][:],
                            start=(i == 0), stop=(i == len(jts) - 1),
                        )
                    go = gpool.tile([128, CW], F32, tag="go")
                    nc.any.tensor_copy(out=go[:], in_=gacc[:])
                    nc.gpsimd.indirect_dma_start(
                        out=out[:],
                        out_offset=bass.IndirectOffsetOnAxis(
                            ap=uoff_b[:, ch * UT + ut:ch * UT + ut + 1], axis=0),
                        in_=go[:],
                        in_offset=None,
                        bounds_check=VS * CH - 1,
                        oob_is_err=False,
                    )

            for ps in range(NP - 1, -1, -1):
                qt_pass(ps)
                chunk_work(2 * ps + 1)
                chunk_work(2 * ps)
    nc.compile()
    return nc


def kernel(x, idx, Wq, Wk):
    x = np.asarray(x, dtype=np.float32)
    idx = np.asarray(idx)
    Wq = np.asarray(Wq, dtype=np.float32)
    Wk = np.asarray(Wk, dtype=np.float32)

    # ---- shared host prep ----
    # xh[ps, c8, p, cc*CW + i] = x[(2*ps+cc)*CW + i, c8*128 + p]
    xh = np.ascontiguousarray(
        x.reshape(NP, 2 * CW, CT, 128).transpose(0, 2, 3, 1))
    wq2 = np.ascontiguousarray(
        (Wq / 256.0).reshape(CT, 128, D).transpose(1, 0, 2).reshape(128, CT * D))
    wk2 = np.ascontiguousarray(
        Wk.reshape(CT, 128, D).transpose(1, 0, 2).reshape(128, CT * D))
    iota = np.broadcast_to(np.arange(CW, dtype=np.float32), (128, CW)).copy()

    # ---- per-core metadata ----
    S, UN, INV = [], [], []
    for p in range(NCORES):
        sp = np.sort(np.where((idx >= VS * p) & (idx < VS * (p + 1)))[0])
        S.append(sp)
        uq, inv = np.unique(idx[sp], return_inverse=True)
        n, nu = len(sp), len(uq)
        # reorder groups by first occurrence (min source s) -> band-diagonal G
        first = np.full(nu, n, np.int64)
        np.minimum.at(first, inv, np.arange(n))
        order = np.argsort(first, kind="stable")      # group old-id by min-s
        rank = np.empty(nu, np.int64)
        rank[order] = np.arange(nu)
        UN.append(uq[order])                          # vocab value by new row
        INV.append(rank[inv])                         # j -> new group row
    J = max(128, int(math.ceil(max(len(s) for s in S) / 128.0)) * 128)
    JT = J // 128
    UT = max(1, int(math.ceil(max(len(u) for u in UN) / 128.0)))

    BIG = 10.0**9
    in_maps = []
    smin = np.full((NCORES, JT), np.inf)
    smax = np.full((NCORES, JT), -np.inf)
    gnz = np.zeros((JT, UT), bool)
    for p in range(NCORES):
        sp, uq, inv = S[p], UN[p], INV[p]
        n = len(sp)
        xs = np.zeros((J, C), np.float32)
        xs[:n] = x[sp]
        xst = np.ascontiguousarray(xs.T.reshape(CT, 128, J))
        s_pad = np.full(J, BIG, np.float64)
        s_pad[:n] = sp
        sadj = np.empty((128, CH * JT), np.float32)
        for ch in range(CH):
            for jt in range(JT):
                sadj[:, ch * JT + jt] = (s_pad[jt * 128:(jt + 1) * 128]
                                         - ch * CW).astype(np.float32)
        g = np.zeros((128, JT * UT * 128), np.float32)
        jj = np.arange(n)
        g[jj % 128, (jj // 128) * UT * 128 + inv] = 1.0
        gnz |= g.reshape(128, JT, UT, 128).sum(axis=(0, 3)) > 0
        uo = np.full((128, CH * UT), OOB, np.int32)
        nu = len(uq)
        gg = np.arange(nu)
        for ch in range(CH):
            uo[gg % 128, ch * UT + gg // 128] = (uq - VS * p) * CH + ch
        in_maps.append({"xh": xh, "wq": wq2, "wk": wk2, "xst": xst,
                        "sadj": sadj, "gct": g, "uoff": uo, "iota": iota})
        for jt in range(JT):
            rows = s_pad[jt * 128:(jt + 1) * 128]
            real = rows[rows < BIG]
            if len(real):
                smin[p, jt] = real.min()
                smax[p, jt] = real.max()

    active, masked = {}, {}
    for ch in range(CH):
        for jt in range(JT):
            a = bool((smin[:, jt] < (ch + 1) * CW).any())
            active[(ch, jt)] = a
            masked[(ch, jt)] = a and bool((smax[:, jt] > ch * CW).any())
    gnnz = {(jt, ut): bool(gnz[jt, ut]) for jt in range(JT) for ut in range(UT)}

    nc = _build(J, JT, UT, active, masked, gnnz)
    res = run_bass_kernel_spmd(nc, in_maps, core_ids=list(range(NCORES)))

    outf = np.zeros((T, V), np.float32)
    for p in range(NCORES):
        blk = res.results[p]["out"].reshape(VS, T)  # [u_local, t]
        outf[:, VS * p:VS * (p + 1)] = blk.T
    return outf


# revision 12
# speedup vs baseline: 1.1377x; 1.1377x over previous
"""HeadQK kernel for trn2: out = segsum_vocab(causal(q @ k.T / 256)) over 8 cores.

Strategy: shard the vocab dimension of the output across the 8 cores.
Core p owns vocab slice [VS*p, VS*(p+1)).  For its slice it needs only the
attention columns s with idx[s] in the slice (~T/8 of them), but all of q.
The output block is produced TRANSPOSED ([VS, T]) so each touched vocab row
is a contiguous DMA; untouched rows remain zero via the runtime's
zero-initialized output buffers.  The segment-sum becomes a small 0/1
group-matrix matmul on the PE array (groups ordered by first source so the
matrix is band-diagonal and zero blocks are skipped); causal masking is an
iota>=s compare on the vector engine.  All matmuls run in float32r.
"""

import math
import sys

import numpy as np

if "/opt/trn_rl_repo" not in sys.path:
    sys.path.insert(0, "/opt/trn_rl_repo")

import concourse.bacc as bacc
import concourse.mybir as mybir
import concourse.tile as tile
from concourse import bass
from concourse.bass_utils import run_bass_kernel_spmd

T, C, D, V = 4096, 1024, 256, 32000
NCORES = 8
VS = V // NCORES        # 4000 vocab slots per core
CH, CW = 8, 512         # t chunks: 8 x 512
NP = 4                  # qT passes, 2 chunks each
CT = C // 128           # 8 contraction tiles
DT = D // 128           # 2 d tiles
F32 = mybir.dt.float32
F32R = mybir.dt.float32r
I32 = mybir.dt.int32
OOB = 10**8


def _build(J, JT, UT, active, masked, gnnz):
    """Build the SPMD program.

    active/masked: dict[(ch, jt)] -> bool
    gnnz: dict[(jt, ut)] -> bool  (G block non-zero on any core)
    """
    nc = bacc.Bacc("TRN2", target_bir_lowering=False, debug=False,
                   num_devices=NCORES)
    xh = nc.dram_tensor("xh", [NP, CT, 128, 2 * CW], F32R, kind="ExternalInput")
    wq = nc.dram_tensor("wq", [128, CT * D], F32R, kind="ExternalInput")
    wk = nc.dram_tensor("wk", [128, CT * D], F32R, kind="ExternalInput")
    xst = nc.dram_tensor("xst", [CT, 128, J], F32R, kind="ExternalInput")
    sadj = nc.dram_tensor("sadj", [128, CH * JT], F32, kind="ExternalInput")
    gct = nc.dram_tensor("gct", [128, JT * UT * 128], F32R, kind="ExternalInput")
    uoff = nc.dram_tensor("uoff", [128, CH * UT], I32, kind="ExternalInput")
    iota = nc.dram_tensor("iota", [128, CW], F32, kind="ExternalInput")
    out = nc.dram_tensor("out", [VS * CH, CW], F32, kind="ExternalOutput")

    with tile.TileContext(nc) as tc:
        with (
            tc.tile_pool(name="const", bufs=1) as cpool,
            tc.tile_pool(name="xbuf", bufs=8) as xpool,
            tc.tile_pool(name="ctm", bufs=2) as mpool,
            tc.tile_pool(name="gout", bufs=3) as gpool,
            tc.tile_pool(name="psq", bufs=4, space="PSUM") as psq,
            tc.tile_pool(name="psc", bufs=2, space="PSUM") as psc,
            tc.tile_pool(name="psg", bufs=2, space="PSUM") as psg,
        ):
            # ---- loads: scalar queue = consts+wk, sync queue = xst+wq ----
            iota_b = cpool.tile([128, CW], F32)
            nc.scalar.dma_start(out=iota_b[:], in_=iota[:])
            sadj_b = cpool.tile([128, CH * JT], F32)
            nc.scalar.dma_start(out=sadj_b[:], in_=sadj[:])
            wk_b = cpool.tile([128, CT * D], F32R)
            nc.scalar.dma_start(out=wk_b[:], in_=wk[:])
            uoff_b = cpool.tile([128, CH * UT], I32)
            nc.scalar.dma_start(out=uoff_b[:], in_=uoff[:])
            gct_b = cpool.tile([128, JT * UT * 128], F32R)
            nc.scalar.dma_start(out=gct_b[:], in_=gct[:])
            xst_b = []
            for c8 in range(CT):
                xt_ = cpool.tile([128, J], F32R, tag=f"xst{c8}")
                nc.sync.dma_start(out=xt_[:], in_=xst[c8])
                xst_b.append(xt_)
            wq_b = cpool.tile([128, CT * D], F32R)
            nc.scalar.dma_start(out=wq_b[:], in_=wq[:])

            # ---- precompute causal masks on the (idle) early DVE ----
            maskt = {}
            for ch in range(CH):
                for jt in range(JT):
                    if masked[(ch, jt)]:
                        mk = cpool.tile([128, CW], F32, tag=f"mk{ch}_{jt}")
                        nc.vector.tensor_tensor(
                            out=mk[:], in0=iota_b[:],
                            in1=sadj_b[:, ch * JT + jt:ch * JT + jt + 1]
                            .to_broadcast([128, CW]),
                            op=mybir.AluOpType.is_ge,
                        )
                        maskt[(ch, jt)] = mk

            # ---- kST[d, j] = Wk.T @ x[S].T (c8-outer: weights amortized) ----
            JW = [(j0, min(512, J - j0)) for j0 in range(0, J, 512)]
            kacc = {}
            for d in range(DT):
                for j0, jw in JW:
                    ka = psq.tile([128, jw], F32, tag="qtp")
                    kacc[(d, j0)] = ka
            for c8 in range(CT):
                for d in range(DT):
                    for j0, jw in JW:
                        nc.tensor.matmul(
                            out=kacc[(d, j0)][:],
                            lhsT=wk_b[:, c8 * D + d * 128:c8 * D + (d + 1) * 128],
                            rhs=xst_b[c8][:, j0:j0 + jw],
                            start=(c8 == 0), stop=(c8 == CT - 1),
                        )
            kst = []
            for d in range(DT):
                kt = cpool.tile([128, J], F32R, tag=f"kst{d}")
                for j0, jw in JW:
                    nc.any.tensor_copy(out=kt[:, j0:j0 + jw], in_=kacc[(d, j0)][:])
                kst.append(kt)

            # ---- qT passes (reverse order) interleaved with chunk work ----
            qt_all = []
            for d in range(DT):
                qa_t = cpool.tile([128, T], F32R, tag=f"qt{d}")
                qt_all.append(qa_t)

            def qt_pass(ps):
                xq = []
                for c8 in range(CT):
                    xq_t = xpool.tile([128, 2 * CW], F32R, tag="xq")
                    nc.sync.dma_start(out=xq_t[:], in_=xh[ps, c8])
                    xq.append(xq_t)
                qacc = {}
                for d in range(DT):
                    for cc in range(2):
                        qa = psq.tile([128, CW], F32, tag="qtp")
                        qacc[(d, cc)] = qa
                for c8 in range(CT):
                    for d in range(DT):
                        for cc in range(2):
                            nc.tensor.matmul(
                                out=qacc[(d, cc)][:],
                                lhsT=wq_b[:, c8 * D + d * 128:c8 * D + (d + 1) * 128],
                                rhs=xq[c8][:, cc * CW:(cc + 1) * CW],
                                start=(c8 == 0), stop=(c8 == CT - 1),
                            )
                for d in range(DT):
                    for cc in range(2):
                        t0 = (2 * ps + cc) * CW
                        nc.any.tensor_copy(out=qt_all[d][:, t0:t0 + CW],
                                           in_=qacc[(d, cc)][:])

            def chunk_work(ch):
                act = [jt for jt in range(JT) if active[(ch, jt)]]
                if not act:
                    return
                ctm = {}
                for jt in act:
                    acc = psc.tile([128, CW], F32, tag="ctp")
                    for d in range(DT):
                        nc.tensor.matmul(
                            out=acc[:],
                            lhsT=kst[d][:, jt * 128:(jt + 1) * 128],
                            rhs=qt_all[d][:, ch * CW:(ch + 1) * CW],
                            start=(d == 0), stop=(d == DT - 1),
                        )
                    cm = mpool.tile([128, CW], F32R, tag=f"cm{jt}")
                    if masked[(ch, jt)]:
                        nc.vector.tensor_tensor(
                            out=cm[:], in0=acc[:], in1=maskt[(ch, jt)][:],
                            op=mybir.AluOpType.mult,
                        )
                    else:
                        nc.any.tensor_copy(out=cm[:], in_=acc[:])
                    ctm[jt] = cm

                for ut in range(UT):
                    jts = [jt for jt in act if gnnz[(jt, ut)]]
                    if not jts:
                        continue
                    gacc = psg.tile([128, CW], F32, tag="gp")
                    for i, jt in enumerate(jts):
                        nc.tensor.matmul(
                            out=gacc[:],
                            lhsT=gct_b[:, jt * UT * 128 + ut * 128:
                                       jt * UT * 128 + (ut + 1) * 128],
                            rhs=ctm[jt][:],
                            start=(i == 0), stop=(i == len(jts) - 1),
                        )
                    go = gpool.tile([128, CW], F32, tag="go")
                    nc.any.tensor_copy(out=go[:], in_=gacc[:])
                    nc.gpsimd.indirect_dma_start(
                        out=out[:],
                        out_offset=bass.IndirectOffsetOnAxis(
                            ap=uoff_b[:, ch * UT + ut:ch * UT + ut + 1], axis=0),
                        in_=go[:],
                        in_offset=None,
                        bounds_check=VS * CH - 1,
                        oob_is_err=False,
                    )

            for ps in range(NP - 1, -1, -1):
                qt_pass(ps)
                chunk_work(2 * ps + 1)
                chunk_work(2 * ps)
    nc.compile()
    return nc


def kernel(x, idx, Wq, Wk):
    x = np.asarray(x, dtype=np.float32)
    idx = np.asarray(idx)
    Wq = np.asarray(Wq, dtype=np.float32)
    Wk = np.asarray(Wk, dtype=np.float32)

    # ---- shared host prep ----
    # xh[ps, c8, p, cc*CW + i] = x[(2*ps+cc)*CW + i, c8*128 + p]
    xh = np.ascontiguousarray(
        x.reshape(NP, 2 * CW, CT, 128).transpose(0, 2, 3, 1))
    wq2 = np.ascontiguousarray(
        (Wq / 256.0).reshape(CT, 128, D).transpose(1, 0, 2).reshape(128, CT * D))
    wk2 = np.ascontiguousarray(
        Wk.reshape(CT, 128, D).transpose(1, 0, 2).reshape(128, CT * D))
    iota = np.broadcast_to(np.arange(CW, dtype=np.float32), (128, CW)).copy()

    # ---- per-core metadata ----
    S, UN, INV = [], [], []
    for p in range(NCORES):
        sp = np.sort(np.where((idx >= VS * p) & (idx < VS * (p + 1)))[0])
        S.append(sp)
        uq, inv = np.unique(idx[sp], return_inverse=True)
        n, nu = len(sp), len(uq)
        # reorder groups by first occurrence (min source s) -> band-diagonal G
        first = np.full(nu, n, np.int64)
        np.minimum.at(first, inv, np.arange(n))
        order = np.argsort(first, kind="stable")      # group old-id by min-s
        rank = np.empty(nu, np.int64)
        rank[order] = np.arange(nu)
        UN.append(uq[order])                          # vocab value by new row
        INV.append(rank[inv])                         # j -> new group row
    J = max(128, int(math.ceil(max(len(s) for s in S) / 128.0)) * 128)
    JT = J // 128
    UT = max(1, int(math.ceil(max(len(u) for u in UN) / 128.0)))

    BIG = 10.0**9
    in_maps = []
    smin = np.full((NCORES, JT), np.inf)
    smax = np.full((NCORES, JT), -np.inf)
    gnz = np.zeros((JT, UT), bool)
    for p in range(NCORES):
        sp, uq, inv = S[p], UN[p], INV[p]
        n = len(sp)
        xs = np.zeros((J, C), np.float32)
        xs[:n] = x[sp]
        xst = np.ascontiguousarray(xs.T.reshape(CT, 128, J))
        s_pad = np.full(J, BIG, np.float64)
        s_pad[:n] = sp
        sadj = np.empty((128, CH * JT), np.float32)
        for ch in range(CH):
            for jt in range(JT):
                sadj[:, ch * JT + jt] = (s_pad[jt * 128:(jt + 1) * 128]
                                         - ch * CW).astype(np.float32)
        g = np.zeros((128, JT * UT * 128), np.float32)
        jj = np.arange(n)
        g[jj % 128, (jj // 128) * UT * 128 + inv] = 1.0
        gnz |= g.reshape(128, JT, UT, 128).sum(axis=(0, 3)) > 0
        uo = np.full((128, CH * UT), OOB, np.int32)
        nu = len(uq)
        gg = np.arange(nu)
        for ch in range(CH):
            uo[gg % 128, ch * UT + gg // 128] = (uq - VS * p) * CH + ch
        in_maps.append({"xh": xh, "wq": wq2, "wk": wk2, "xst": xst,
                        "sadj": sadj, "gct": g, "uoff": uo, "iota": iota})
        for jt in range(JT):
            rows = s_pad[jt * 128:(jt + 1) * 128]
            real = rows[rows < BIG]
            if len(real):
                smin[p, jt] = real.min()
                smax[p, jt] = real.max()

    active, masked = {}, {}
    for ch in range(CH):
        for jt in range(JT):
            a = bool((smin[:, jt] < (ch + 1) * CW).any())
            active[(ch, jt)] = a
            masked[(ch, jt)] = a and bool((smax[:, jt] > ch * CW).any())
    gnnz = {(jt, ut): bool(gnz[jt, ut]) for jt in range(JT) for ut in range(UT)}

    nc = _build(J, JT, UT, active, masked, gnnz)
    res = run_bass_kernel_spmd(nc, in_maps, core_ids=list(range(NCORES)))

    outf = np.zeros((T, V), np.float32)
    for p in range(NCORES):
        blk = res.results[p]["out"].reshape(VS, T)  # [u_local, t]
        outf[:, VS * p:VS * (p + 1)] = blk.T
    return outf


# revision 13
# speedup vs baseline: 1.3571x; 1.1928x over previous
"""HeadQK kernel for trn2: out = segsum_vocab(causal(q @ k.T / 256)) over 8 cores.

Strategy: shard the vocab dimension of the output across the 8 cores.
Core p owns vocab slice [VS*p, VS*(p+1)).  For its slice it needs only the
attention columns s with idx[s] in the slice (~T/8 of them), but all of q.
The output block is produced TRANSPOSED ([VS, T]) so each touched vocab row
is a contiguous DMA; untouched rows remain zero via the runtime's
zero-initialized output buffers.  The segment-sum becomes a small 0/1
group-matrix matmul on the PE array (groups ordered by first source so the
matrix is band-diagonal and zero blocks are skipped); causal masking is an
iota>=s compare on the vector engine.  All matmuls run in float32r.
"""

import math
import sys

import numpy as np

if "/opt/trn_rl_repo" not in sys.path:
    sys.path.insert(0, "/opt/trn_rl_repo")

import concourse.bacc as bacc
import concourse.mybir as mybir
import concourse.tile as tile
from concourse import bass
from concourse.bass_utils import run_bass_kernel_spmd

T, C, D, V = 4096, 1024, 256, 32000
NCORES = 8
VS = V // NCORES        # 4000 vocab slots per core
CH, CW = 8, 512         # t chunks: 8 x 512
NP = 4                  # qT passes, 2 chunks each
CT = C // 128           # 8 contraction tiles
DT = D // 128           # 2 d tiles
F32 = mybir.dt.float32
F32R = mybir.dt.float32r
I32 = mybir.dt.int32
OOB = 10**8


def _build(J, JT, UT, active, masked, gnnz):
    """Build the SPMD program.

    active/masked: dict[(ch, jt)] -> bool
    gnnz: dict[(jt, ut)] -> bool  (G block non-zero on any core)
    """
    nc = bacc.Bacc("TRN2", target_bir_lowering=False, debug=False,
                   num_devices=NCORES)
    xh = nc.dram_tensor("xh", [NP, CT, 128, 2 * CW], F32R, kind="ExternalInput")
    wq = nc.dram_tensor("wq", [128, CT * D], F32R, kind="ExternalInput")
    wk = nc.dram_tensor("wk", [128, CT * D], F32R, kind="ExternalInput")
    xst = nc.dram_tensor("xst", [CT, 128, J], F32R, kind="ExternalInput")
    sadj = nc.dram_tensor("sadj", [128, CH * JT], F32, kind="ExternalInput")
    gct = nc.dram_tensor("gct", [128, JT * UT * 128], F32R, kind="ExternalInput")
    uoff = nc.dram_tensor("uoff", [128, (CH + NP) * UT], I32, kind="ExternalInput")
    iota = nc.dram_tensor("iota", [128, CW], F32, kind="ExternalInput")
    out = nc.dram_tensor("out", [VS * CH, CW], F32, kind="ExternalOutput")

    with tile.TileContext(nc) as tc:
        with (
            tc.tile_pool(name="const", bufs=1) as cpool,
            tc.tile_pool(name="xbuf", bufs=10) as xpool,
            tc.tile_pool(name="ctm", bufs=2) as mpool,
            tc.tile_pool(name="gout", bufs=3) as gpool,
            tc.tile_pool(name="psq", bufs=2, space="PSUM") as psq,
            tc.tile_pool(name="psc", bufs=4, space="PSUM") as psc,
            tc.tile_pool(name="psg", bufs=2, space="PSUM") as psg,
        ):
            # ---- loads: scalar queue = consts+wk, sync queue = xst+wq ----
            iota_b = cpool.tile([128, CW], F32)
            nc.scalar.dma_start(out=iota_b[:], in_=iota[:])
            sadj_b = cpool.tile([128, CH * JT], F32)
            nc.scalar.dma_start(out=sadj_b[:], in_=sadj[:])
            wk_b = cpool.tile([128, CT * D], F32R)
            nc.scalar.dma_start(out=wk_b[:], in_=wk[:])
            uoff_b = cpool.tile([128, (CH + NP) * UT], I32)
            nc.scalar.dma_start(out=uoff_b[:], in_=uoff[:])
            gct_b = cpool.tile([128, JT * UT * 128], F32R)
            nc.scalar.dma_start(out=gct_b[:], in_=gct[:])
            xst_b = []
            for c8 in range(CT):
                xt_ = cpool.tile([128, J], F32R, tag=f"xst{c8}")
                nc.sync.dma_start(out=xt_[:], in_=xst[c8])
                xst_b.append(xt_)
            wq_b = cpool.tile([128, CT * D], F32R)
            nc.scalar.dma_start(out=wq_b[:], in_=wq[:])

            # ---- kST[d, j] = Wk.T @ x[S].T (c8-outer: weights amortized) ----
            JW = [(j0, min(512, J - j0)) for j0 in range(0, J, 512)]
            kst = []
            for d in range(DT):
                kacc = {}
                for j0, jw in JW:
                    ka = psq.tile([128, jw], F32, tag="qtp")
                    kacc[j0] = ka
                for c8 in range(CT):
                    for j0, jw in JW:
                        nc.tensor.matmul(
                            out=kacc[j0][:],
                            lhsT=wk_b[:, c8 * D + d * 128:c8 * D + (d + 1) * 128],
                            rhs=xst_b[c8][:, j0:j0 + jw],
                            start=(c8 == 0), stop=(c8 == CT - 1),
                        )
                kt = cpool.tile([128, J], F32R, tag=f"kst{d}")
                for j0, jw in JW:
                    nc.any.tensor_copy(out=kt[:, j0:j0 + jw], in_=kacc[j0][:])
                kst.append(kt)

            # ---- qT passes (reverse order) interleaved with chunk work ----
            qt_all = []
            for d in range(DT):
                qa_t = cpool.tile([128, T], F32R, tag=f"qt{d}")
                qt_all.append(qa_t)

            def qt_pass(ps):
                xq = []
                for c8 in range(CT):
                    xq_t = xpool.tile([128, 2 * CW], F32R, tag="xq")
                    nc.sync.dma_start(out=xq_t[:], in_=xh[ps, c8])
                    xq.append(xq_t)
                for d in range(DT):
                    qacc = {}
                    for cc in range(2):
                        qa = psq.tile([128, CW], F32, tag="qtp")
                        qacc[cc] = qa
                    for c8 in range(CT):
                        for cc in range(2):
                            nc.tensor.matmul(
                                out=qacc[cc][:],
                                lhsT=wq_b[:, c8 * D + d * 128:c8 * D + (d + 1) * 128],
                                rhs=xq[c8][:, cc * CW:(cc + 1) * CW],
                                start=(c8 == 0), stop=(c8 == CT - 1),
                            )
                    for cc in range(2):
                        t0 = (2 * ps + cc) * CW
                        nc.any.tensor_copy(out=qt_all[d][:, t0:t0 + CW],
                                           in_=qacc[cc][:])

            def chunk_work(ch, gouts):
                ps, half = ch // 2, ch % 2
                act = [jt for jt in range(JT) if active[(ch, jt)]]
                if not act:
                    return
                ctm = {}
                for jt in act:
                    acc = psc.tile([128, CW], F32, tag="ctp")
                    for d in range(DT):
                        nc.tensor.matmul(
                            out=acc[:],
                            lhsT=kst[d][:, jt * 128:(jt + 1) * 128],
                            rhs=qt_all[d][:, ch * CW:(ch + 1) * CW],
                            start=(d == 0), stop=(d == DT - 1),
                        )
                    cm = mpool.tile([128, CW], F32R, tag=f"cm{jt}")
                    if masked[(ch, jt)]:
                        mk = mpool.tile([128, CW], F32, tag="mask")
                        nc.vector.tensor_tensor(
                            out=mk[:], in0=iota_b[:],
                            in1=sadj_b[:, ch * JT + jt:ch * JT + jt + 1]
                            .to_broadcast([128, CW]),
                            op=mybir.AluOpType.is_ge,
                        )
                        nc.vector.tensor_tensor(
                            out=cm[:], in0=acc[:], in1=mk[:],
                            op=mybir.AluOpType.mult,
                        )
                    else:
                        nc.any.tensor_copy(out=cm[:], in_=acc[:])
                    ctm[jt] = cm

                for ut in range(UT):
                    jts = [jt for jt in act if gnnz[(jt, ut)]]
                    if not jts:
                        continue
                    gacc = psg.tile([128, CW], F32, tag="gp")
                    for i, jt in enumerate(jts):
                        nc.tensor.matmul(
                            out=gacc[:],
                            lhsT=gct_b[:, jt * UT * 128 + ut * 128:
                                       jt * UT * 128 + (ut + 1) * 128],
                            rhs=ctm[jt][:],
                            start=(i == 0), stop=(i == len(jts) - 1),
                        )
                    if (ps, ut) in gouts:
                        go = gouts[(ps, ut)]
                        nc.any.tensor_copy(out=go[:, half * CW:(half + 1) * CW],
                                           in_=gacc[:])
                    else:
                        go1 = gpool.tile([128, CW], F32, tag="go1")
                        nc.any.tensor_copy(out=go1[:], in_=gacc[:])
                        nc.gpsimd.indirect_dma_start(
                            out=out[:],
                            out_offset=bass.IndirectOffsetOnAxis(
                                ap=uoff_b[:, ch * UT + ut:ch * UT + ut + 1],
                                axis=0),
                            in_=go1[:],
                            in_offset=None,
                            bounds_check=VS * CH - 1,
                            oob_is_err=False,
                        )

            def contrib(ch, ut):
                return any(active[(ch, jt)] and gnnz[(jt, ut)]
                           for jt in range(JT))

            out_pair = out[:].rearrange("(a b) w -> a (b w)", b=2)
            for ps in range(NP - 1, -1, -1):
                qt_pass(ps)
                gouts = {}
                for ut in range(UT):
                    if contrib(2 * ps, ut) and contrib(2 * ps + 1, ut):
                        gp_t = gpool.tile([128, 2 * CW], F32, tag="go")
                        gouts[(ps, ut)] = gp_t
                chunk_work(2 * ps + 1, gouts)
                chunk_work(2 * ps, gouts)
                for ut in range(UT):
                    if (ps, ut) in gouts:
                        nc.gpsimd.indirect_dma_start(
                            out=out_pair,
                            out_offset=bass.IndirectOffsetOnAxis(
                                ap=uoff_b[:, CH * UT + ps * UT + ut:
                                          CH * UT + ps * UT + ut + 1], axis=0),
                            in_=gouts[(ps, ut)][:],
                            in_offset=None,
                            bounds_check=VS * NP - 1,
                            oob_is_err=False,
                        )
    nc.compile()
    return nc


def kernel(x, idx, Wq, Wk):
    x = np.asarray(x, dtype=np.float32)
    idx = np.asarray(idx)
    Wq = np.asarray(Wq, dtype=np.float32)
    Wk = np.asarray(Wk, dtype=np.float32)

    # ---- shared host prep ----
    # xh[ps, c8, p, cc*CW + i] = x[(2*ps+cc)*CW + i, c8*128 + p]
    xh = np.ascontiguousarray(
        x.reshape(NP, 2 * CW, CT, 128).transpose(0, 2, 3, 1))
    wq2 = np.ascontiguousarray(
        (Wq / 256.0).reshape(CT, 128, D).transpose(1, 0, 2).reshape(128, CT * D))
    wk2 = np.ascontiguousarray(
        Wk.reshape(CT, 128, D).transpose(1, 0, 2).reshape(128, CT * D))
    iota = np.broadcast_to(np.arange(CW, dtype=np.float32), (128, CW)).copy()

    # ---- per-core metadata ----
    S, UN, INV = [], [], []
    for p in range(NCORES):
        sp = np.sort(np.where((idx >= VS * p) & (idx < VS * (p + 1)))[0])
        S.append(sp)
        uq, inv = np.unique(idx[sp], return_inverse=True)
        n, nu = len(sp), len(uq)
        # reorder groups by first occurrence (min source s) -> band-diagonal G
        first = np.full(nu, n, np.int64)
        np.minimum.at(first, inv, np.arange(n))
        order = np.argsort(first, kind="stable")      # group old-id by min-s
        rank = np.empty(nu, np.int64)
        rank[order] = np.arange(nu)
        UN.append(uq[order])                          # vocab value by new row
        INV.append(rank[inv])                         # j -> new group row
    J = max(128, int(math.ceil(max(len(s) for s in S) / 128.0)) * 128)
    JT = J // 128
    UT = max(1, int(math.ceil(max(len(u) for u in UN) / 128.0)))

    BIG = 10.0**9
    in_maps = []
    smin = np.full((NCORES, JT), np.inf)
    smax = np.full((NCORES, JT), -np.inf)
    gnz = np.zeros((JT, UT), bool)
    for p in range(NCORES):
        sp, uq, inv = S[p], UN[p], INV[p]
        n = len(sp)
        xs = np.zeros((J, C), np.float32)
        xs[:n] = x[sp]
        xst = np.ascontiguousarray(xs.T.reshape(CT, 128, J))
        s_pad = np.full(J, BIG, np.float64)
        s_pad[:n] = sp
        sadj = np.empty((128, CH * JT), np.float32)
        for ch in range(CH):
            for jt in range(JT):
                sadj[:, ch * JT + jt] = (s_pad[jt * 128:(jt + 1) * 128]
                                         - ch * CW).astype(np.float32)
        g = np.zeros((128, JT * UT * 128), np.float32)
        jj = np.arange(n)
        g[jj % 128, (jj // 128) * UT * 128 + inv] = 1.0
        gnz |= g.reshape(128, JT, UT, 128).sum(axis=(0, 3)) > 0
        uo = np.full((128, (CH + NP) * UT), OOB, np.int32)
        nu = len(uq)
        gg = np.arange(nu)
        for ch in range(CH):
            uo[gg % 128, ch * UT + gg // 128] = (uq - VS * p) * CH + ch
        for ps in range(NP):
            uo[gg % 128, CH * UT + ps * UT + gg // 128] = (uq - VS * p) * NP + ps
        in_maps.append({"xh": xh, "wq": wq2, "wk": wk2, "xst": xst,
                        "sadj": sadj, "gct": g, "uoff": uo, "iota": iota})
        for jt in range(JT):
            rows = s_pad[jt * 128:(jt + 1) * 128]
            real = rows[rows < BIG]
            if len(real):
                smin[p, jt] = real.min()
                smax[p, jt] = real.max()

    active, masked = {}, {}
    for ch in range(CH):
        for jt in range(JT):
            a = bool((smin[:, jt] < (ch + 1) * CW).any())
            active[(ch, jt)] = a
            masked[(ch, jt)] = a and bool((smax[:, jt] > ch * CW).any())
    gnnz = {(jt, ut): bool(gnz[jt, ut]) for jt in range(JT) for ut in range(UT)}

    nc = _build(J, JT, UT, active, masked, gnnz)
    res = run_bass_kernel_spmd(nc, in_maps, core_ids=list(range(NCORES)))

    outf = np.zeros((T, V), np.float32)
    for p in range(NCORES):
        blk = res.results[p]["out"].reshape(VS, T)  # [u_local, t]
        outf[:, VS * p:VS * (p + 1)] = blk.T
    return outf


# revision 14
# speedup vs baseline: 1.3788x; 1.0160x over previous
"""HeadQK kernel for trn2: out = segsum_vocab(causal(q @ k.T / 256)) over 8 cores.

Strategy: shard the vocab dimension of the output across the 8 cores.
Core p owns vocab slice [VS*p, VS*(p+1)).  For its slice it needs only the
attention columns s with idx[s] in the slice (~T/8 of them), but all of q.
The output block is produced TRANSPOSED ([VS, T]) so each touched vocab row
is a contiguous DMA; untouched rows remain zero via the runtime's
zero-initialized output buffers.  The segment-sum becomes a small 0/1
group-matrix matmul on the PE array (groups ordered by first source so the
matrix is band-diagonal and zero blocks are skipped); causal masking is an
iota>=s compare on the vector engine.  All matmuls run in float32r.
"""

import math
import sys

import numpy as np

if "/opt/trn_rl_repo" not in sys.path:
    sys.path.insert(0, "/opt/trn_rl_repo")

import concourse.bacc as bacc
import concourse.mybir as mybir
import concourse.tile as tile
from concourse import bass
from concourse.bass_utils import run_bass_kernel_spmd

T, C, D, V = 4096, 1024, 256, 32000
NCORES = 8
VS = V // NCORES        # 4000 vocab slots per core
CH, CW = 8, 512         # t chunks: 8 x 512
NP = 4                  # qT passes, 2 chunks each
CT = C // 128           # 8 contraction tiles
DT = D // 128           # 2 d tiles
F32 = mybir.dt.float32
F32R = mybir.dt.float32r
I32 = mybir.dt.int32
OOB = 10**8


def _build(J, JT, UT, active, masked, gnnz):
    """Build the SPMD program.

    active/masked: dict[(ch, jt)] -> bool
    gnnz: dict[(jt, ut)] -> bool  (G block non-zero on any core)
    """
    nc = bacc.Bacc("TRN2", target_bir_lowering=False, debug=False,
                   num_devices=NCORES)
    xh = nc.dram_tensor("xh", [NP, CT, 128, 2 * CW], F32R, kind="ExternalInput")
    wq = nc.dram_tensor("wq", [128, CT * D], F32R, kind="ExternalInput")
    wk = nc.dram_tensor("wk", [128, CT * D], F32R, kind="ExternalInput")
    xst = nc.dram_tensor("xst", [CT, 128, J], F32R, kind="ExternalInput")
    sadj = nc.dram_tensor("sadj", [128, CH * JT], F32, kind="ExternalInput")
    gct = nc.dram_tensor("gct", [128, JT * UT * 128], F32R, kind="ExternalInput")
    uoff = nc.dram_tensor("uoff", [128, (CH + NP) * UT], I32, kind="ExternalInput")
    iota = nc.dram_tensor("iota", [128, CW], F32, kind="ExternalInput")
    out = nc.dram_tensor("out", [VS * CH, CW], F32, kind="ExternalOutput")

    with tile.TileContext(nc) as tc:
        with (
            tc.tile_pool(name="const", bufs=1) as cpool,
            tc.tile_pool(name="xbuf", bufs=10) as xpool,
            tc.tile_pool(name="ctm", bufs=2) as mpool,
            tc.tile_pool(name="gout", bufs=5) as gpool,
            tc.tile_pool(name="gout1", bufs=4) as gpool1,
            tc.tile_pool(name="psq", bufs=2, space="PSUM") as psq,
            tc.tile_pool(name="psc", bufs=4, space="PSUM") as psc,
            tc.tile_pool(name="psg", bufs=2, space="PSUM") as psg,
        ):
            # ---- loads: scalar queue = consts+wk, sync queue = xst+wq ----
            wk_b = cpool.tile([128, CT * D], F32R)
            nc.scalar.dma_start(out=wk_b[:], in_=wk[:])
            iota_b = cpool.tile([128, CW], F32)
            nc.scalar.dma_start(out=iota_b[:], in_=iota[:])
            sadj_b = cpool.tile([128, CH * JT], F32)
            nc.scalar.dma_start(out=sadj_b[:], in_=sadj[:])
            uoff_b = cpool.tile([128, (CH + NP) * UT], I32)
            nc.scalar.dma_start(out=uoff_b[:], in_=uoff[:])
            gct_b = cpool.tile([128, JT * UT * 128], F32R)
            nc.scalar.dma_start(out=gct_b[:], in_=gct[:])
            xst_b = []
            for c8 in range(CT):
                xt_ = cpool.tile([128, J], F32R, tag=f"xst{c8}")
                nc.sync.dma_start(out=xt_[:], in_=xst[c8])
                xst_b.append(xt_)
            wq_b = cpool.tile([128, CT * D], F32R)
            nc.scalar.dma_start(out=wq_b[:], in_=wq[:])

            # ---- kST[d, j] = Wk.T @ x[S].T (c8-outer: weights amortized) ----
            JW = [(j0, min(512, J - j0)) for j0 in range(0, J, 512)]
            kst = []
            for d in range(DT):
                kacc = {}
                for j0, jw in JW:
                    ka = psq.tile([128, jw], F32, tag="qtp")
                    kacc[j0] = ka
                for c8 in range(CT):
                    for j0, jw in JW:
                        nc.tensor.matmul(
                            out=kacc[j0][:],
                            lhsT=wk_b[:, c8 * D + d * 128:c8 * D + (d + 1) * 128],
                            rhs=xst_b[c8][:, j0:j0 + jw],
                            start=(c8 == 0), stop=(c8 == CT - 1),
                        )
                kt = cpool.tile([128, J], F32R, tag=f"kst{d}")
                for j0, jw in JW:
                    nc.any.tensor_copy(out=kt[:, j0:j0 + jw], in_=kacc[j0][:])
                kst.append(kt)

            # ---- qT passes (reverse order) interleaved with chunk work ----
            qt_all = []
            for d in range(DT):
                qa_t = cpool.tile([128, T], F32R, tag=f"qt{d}")
                qt_all.append(qa_t)

            def qt_pass(ps):
                xq = []
                for c8 in range(CT):
                    xq_t = xpool.tile([128, 2 * CW], F32R, tag="xq")
                    nc.sync.dma_start(out=xq_t[:], in_=xh[ps, c8])
                    xq.append(xq_t)
                for d in range(DT):
                    qacc = {}
                    for cc in range(2):
                        qa = psq.tile([128, CW], F32, tag="qtp")
                        qacc[cc] = qa
                    for c8 in range(CT):
                        for cc in range(2):
                            nc.tensor.matmul(
                                out=qacc[cc][:],
                                lhsT=wq_b[:, c8 * D + d * 128:c8 * D + (d + 1) * 128],
                                rhs=xq[c8][:, cc * CW:(cc + 1) * CW],
                                start=(c8 == 0), stop=(c8 == CT - 1),
                            )
                    for cc in range(2):
                        t0 = (2 * ps + cc) * CW
                        nc.any.tensor_copy(out=qt_all[d][:, t0:t0 + CW],
                                           in_=qacc[cc][:])

            def chunk_work(ch, gouts):
                ps, half = ch // 2, ch % 2
                act = [jt for jt in range(JT) if active[(ch, jt)]]
                if not act:
                    return
                ctm = {}
                for jt in act:
                    acc = psc.tile([128, CW], F32, tag="ctp")
                    for d in range(DT):
                        nc.tensor.matmul(
                            out=acc[:],
                            lhsT=kst[d][:, jt * 128:(jt + 1) * 128],
                            rhs=qt_all[d][:, ch * CW:(ch + 1) * CW],
                            start=(d == 0), stop=(d == DT - 1),
                        )
                    cm = mpool.tile([128, CW], F32R, tag=f"cm{jt}")
                    if masked[(ch, jt)]:
                        mk = mpool.tile([128, CW], F32, tag="mask")
                        nc.vector.tensor_tensor(
                            out=mk[:], in0=iota_b[:],
                            in1=sadj_b[:, ch * JT + jt:ch * JT + jt + 1]
                            .to_broadcast([128, CW]),
                            op=mybir.AluOpType.is_ge,
                        )
                        nc.vector.tensor_tensor(
                            out=cm[:], in0=acc[:], in1=mk[:],
                            op=mybir.AluOpType.mult,
                        )
                    else:
                        nc.any.tensor_copy(out=cm[:], in_=acc[:])
                    ctm[jt] = cm

                for ut in range(UT):
                    jts = [jt for jt in act if gnnz[(jt, ut)]]
                    if not jts:
                        continue
                    gacc = psg.tile([128, CW], F32, tag="gp")
                    for i, jt in enumerate(jts):
                        nc.tensor.matmul(
                            out=gacc[:],
                            lhsT=gct_b[:, jt * UT * 128 + ut * 128:
                                       jt * UT * 128 + (ut + 1) * 128],
                            rhs=ctm[jt][:],
                            start=(i == 0), stop=(i == len(jts) - 1),
                        )
                    if (ps, ut) in gouts:
                        go = gouts[(ps, ut)]
                        nc.any.tensor_copy(out=go[:, half * CW:(half + 1) * CW],
                                           in_=gacc[:])
                    else:
                        go1 = gpool1.tile([128, CW], F32, tag="go1")
                        nc.any.tensor_copy(out=go1[:], in_=gacc[:])
                        nc.gpsimd.indirect_dma_start(
                            out=out[:],
                            out_offset=bass.IndirectOffsetOnAxis(
                                ap=uoff_b[:, ch * UT + ut:ch * UT + ut + 1],
                                axis=0),
                            in_=go1[:],
                            in_offset=None,
                            bounds_check=VS * CH - 1,
                            oob_is_err=False,
                        )

            def contrib(ch, ut):
                return any(active[(ch, jt)] and gnnz[(jt, ut)]
                           for jt in range(JT))

            out_pair = out[:].rearrange("(a b) w -> a (b w)", b=2)
            for ps in range(NP - 1, -1, -1):
                qt_pass(ps)
                gouts = {}
                for ut in range(UT):
                    if contrib(2 * ps, ut) and contrib(2 * ps + 1, ut):
                        gp_t = gpool.tile([128, 2 * CW], F32, tag="go")
                        gouts[(ps, ut)] = gp_t
                chunk_work(2 * ps + 1, gouts)
                chunk_work(2 * ps, gouts)
                for ut in range(UT):
                    if (ps, ut) in gouts:
                        nc.gpsimd.indirect_dma_start(
                            out=out_pair,
                            out_offset=bass.IndirectOffsetOnAxis(
                                ap=uoff_b[:, CH * UT + ps * UT + ut:
                                          CH * UT + ps * UT + ut + 1], axis=0),
                            in_=gouts[(ps, ut)][:],
                            in_offset=None,
                            bounds_check=VS * NP - 1,
                            oob_is_err=False,
                        )
    nc.compile()
    return nc


def kernel(x, idx, Wq, Wk):
    x = np.asarray(x, dtype=np.float32)
    idx = np.asarray(idx)
    Wq = np.asarray(Wq, dtype=np.float32)
    Wk = np.asarray(Wk, dtype=np.float32)

    # ---- shared host prep ----
    # xh[ps, c8, p, cc*CW + i] = x[(2*ps+cc)*CW + i, c8*128 + p]
    xh = np.ascontiguousarray(
        x.reshape(NP, 2 * CW, CT, 128).transpose(0, 2, 3, 1))
    wq2 = np.ascontiguousarray(
        (Wq / 256.0).reshape(CT, 128, D).transpose(1, 0, 2).reshape(128, CT * D))
    wk2 = np.ascontiguousarray(
        Wk.reshape(CT, 128, D).transpose(1, 0, 2).reshape(128, CT * D))
    iota = np.broadcast_to(np.arange(CW, dtype=np.float32), (128, CW)).copy()

    # ---- per-core metadata ----
    S, UN, INV = [], [], []
    for p in range(NCORES):
        sp = np.sort(np.where((idx >= VS * p) & (idx < VS * (p + 1)))[0])
        S.append(sp)
        uq, inv = np.unique(idx[sp], return_inverse=True)
        n, nu = len(sp), len(uq)
        # reorder groups by first occurrence (min source s) -> band-diagonal G
        first = np.full(nu, n, np.int64)
        np.minimum.at(first, inv, np.arange(n))
        order = np.argsort(first, kind="stable")      # group old-id by min-s
        rank = np.empty(nu, np.int64)
        rank[order] = np.arange(nu)
        UN.append(uq[order])                          # vocab value by new row
        INV.append(rank[inv])                         # j -> new group row
    J = max(128, int(math.ceil(max(len(s) for s in S) / 128.0)) * 128)
    JT = J // 128
    UT = max(1, int(math.ceil(max(len(u) for u in UN) / 128.0)))

    BIG = 10.0**9
    in_maps = []
    smin = np.full((NCORES, JT), np.inf)
    smax = np.full((NCORES, JT), -np.inf)
    gnz = np.zeros((JT, UT), bool)
    for p in range(NCORES):
        sp, uq, inv = S[p], UN[p], INV[p]
        n = len(sp)
        xs = np.zeros((J, C), np.float32)
        xs[:n] = x[sp]
        xst = np.ascontiguousarray(xs.T.reshape(CT, 128, J))
        s_pad = np.full(J, BIG, np.float64)
        s_pad[:n] = sp
        sadj = np.empty((128, CH * JT), np.float32)
        for ch in range(CH):
            for jt in range(JT):
                sadj[:, ch * JT + jt] = (s_pad[jt * 128:(jt + 1) * 128]
                                         - ch * CW).astype(np.float32)
        g = np.zeros((128, JT * UT * 128), np.float32)
        jj = np.arange(n)
        g[jj % 128, (jj // 128) * UT * 128 + inv] = 1.0
        gnz |= g.reshape(128, JT, UT, 128).sum(axis=(0, 3)) > 0
        uo = np.full((128, (CH + NP) * UT), OOB, np.int32)
        nu = len(uq)
        gg = np.arange(nu)
        for ch in range(CH):
            uo[gg % 128, ch * UT + gg // 128] = (uq - VS * p) * CH + ch
        for ps in range(NP):
            uo[gg % 128, CH * UT + ps * UT + gg // 128] = (uq - VS * p) * NP + ps
        in_maps.append({"xh": xh, "wq": wq2, "wk": wk2, "xst": xst,
                        "sadj": sadj, "gct": g, "uoff": uo, "iota": iota})
        for jt in range(JT):
            rows = s_pad[jt * 128:(jt + 1) * 128]
            real = rows[rows < BIG]
            if len(real):
                smin[p, jt] = real.min()
                smax[p, jt] = real.max()

    active, masked = {}, {}
    for ch in range(CH):
        for jt in range(JT):
            a = bool((smin[:, jt] < (ch + 1) * CW).any())
            active[(ch, jt)] = a
            masked[(ch, jt)] = a and bool((smax[:, jt] > ch * CW).any())
    gnnz = {(jt, ut): bool(gnz[jt, ut]) for jt in range(JT) for ut in range(UT)}

    nc = _build(J, JT, UT, active, masked, gnnz)
    res = run_bass_kernel_spmd(nc, in_maps, core_ids=list(range(NCORES)))

    outf = np.zeros((T, V), np.float32)
    for p in range(NCORES):
        blk = res.results[p]["out"].reshape(VS, T)  # [u_local, t]
        outf[:, VS * p:VS * (p + 1)] = blk.T
    return outf
